# revision 1
# baseline (speedup 1.0000x reference)
"""Trainium2 Bass kernel for nn_Conv2d_int8_STE.

Reference computation (per the oracle):
  sx = max|x|/127 ; qx = round(x/sx)          (levels in [-127,127])
  sw = max|w|/127 ; qw = round(w/sw)
  out = conv2d(qx, qw, pad=1) * (sx*sw) + bias
The LUT input is the exact int8 product table lut[i,j]=(i-128)*(j-128),
so the LUT-gather-sum conv is mathematically an ordinary convolution over
the integer levels.  Integer levels |q|<=127 are exact in bf16 and their
products/sums are exact in fp32 PSUM accumulation, so the tensor-engine
matmul path reproduces the reference essentially bit-exactly.

Sharding: data-parallel over batch B=8 across the 8 NeuronCores (one
image per core).  Weights/bias/scales replicated.

Per-core device pipeline, software-pipelined over CHUNKS row-chunks:
  - x is DMAed from DRAM three times into partition groups 0-31/32-63/
    64-95 (the conv's kw taps), in parallel on the SP/ACT/Pool queues.
  - quantize with the fp32 magic-number trick (round-to-nearest-even):
      u = x*inv_sx + 1.5*2^23        (one op over all 96 partitions)
      q = u - 1.5*2^23 -> bf16       (per kw-group, placing each group
                                      column-shifted into a zero-bordered
                                      [*, 34x34] padded buffer)
  - per chunk: 3 accumulating matmuls over kh (lhsT = quantized weights
    [K=96, 32], rhs = padded window rows) -> PSUM.
  - epilogue: out = psum * (sx*sw) + bias; DMA out per chunk.
"""

import os
import sys

for _p in ("/opt/trn_rl_repo", "/root/.axon_site/_ro/trn_rl_repo"):
    if os.path.isdir(_p) and _p not in sys.path:
        sys.path.insert(0, _p)

import numpy as np

import concourse.bass as bass
import concourse.tile as tile
from concourse import bacc, mybir
from concourse.bass_utils import run_bass_kernel_spmd

F32 = mybir.dt.float32
BF16 = mybir.dt.bfloat16
MULT = mybir.AluOpType.mult
ADD = mybir.AluOpType.add

B, CIN, H, W = 8, 32, 32, 32
COUT, KH, KW = 32, 3, 3
PW = W + 2          # padded width 34
PH = H + 2
PHW = PW * PH       # 34*34 = 1156
OHW = H * W
K96 = KW * CIN
MAGIC = float(np.float32(1.5 * 2**23))

N_CORES = 8
CHUNKS = int(os.environ.get("KCHUNKS", "4"))
_CACHE = {}


def _chunk_rows():
    """Disjoint x-row ranges per chunk; chunk 0 additionally carries the
    top halo row each later chunk's first output row needs."""
    R = H // CHUNKS
    xr0 = [0] + [R * q + 1 for q in range(1, CHUNKS)]
    xnr = [R + 1] + [R] * (CHUNKS - 2) + [R - 1]
    return R, xr0, xnr


def _build_program(inv_sx, inv_sw, s_out):
    nc = bacc.Bacc("TRN2", target_bir_lowering=False, debug=False,
                   num_devices=N_CORES)

    x_d = nc.dram_tensor("x", [CIN, OHW], F32, kind="ExternalInput")
    wt_d = nc.dram_tensor("wt", [K96, KH * COUT], BF16, kind="ExternalInput")
    aux_d = nc.dram_tensor("aux", [COUT, 1], F32, kind="ExternalInput")
    out_d = nc.dram_tensor("out", [COUT, OHW], F32, kind="ExternalOutput")

    R, XR0, XNR = _chunk_rows()

    with tile.TileContext(nc) as tc:
        with (
            tc.tile_pool(name="sbuf", bufs=1) as pool,
            tc.tile_pool(name="psum", bufs=1, space="PSUM") as psum,
        ):
            aux = pool.tile([COUT, 1], F32)
            bias = aux[:, 0:1]
            wq = pool.tile([K96, KH * COUT], BF16)

            # quantized, padded, kw-shifted input; K=96 on partitions
            p96 = pool.tile([K96, PHW], BF16)
            p96_rows = p96[:].rearrange("p (r c) -> p r c", c=PW)
            # zero the border cells each kw-group's matmul reads but whose
            # quantize pass never writes (top/bottom pad rows; g0 left pad
            # column; g2 right pad column)
            nc.vector.memset(p96[:, 0:W], 0.0)
            nc.vector.memset(p96[:, (PH - 1) * PW:(PH - 1) * PW + W], 0.0)
            nc.vector.memset(
                p96[0:CIN, PW:PW + PW * H].rearrange(
                    "p (r c) -> p r c", c=PW)[:, :, 0:1], 0.0)
            nc.vector.memset(
                p96[2 * CIN:3 * CIN, PW + 31:PW + 31 + PW * H].rearrange(
                    "p (r c) -> p r c", c=PW)[:, :, 0:1], 0.0)

            # raw x, replicated into the 3 partition groups (parallel DMAs
            # on the three DMA-capable queues); weight/bias DMAs slotted on
            # SP between chunk loads.
            xr = [pool.tile([K96, XNR[c] * W], F32, name=f"xr{c}",
                            tag=f"xr{c}") for c in range(CHUNKS)]
            for c in range(CHUNKS):
                src = x_d.ap()[:, XR0[c] * W:(XR0[c] + XNR[c]) * W]
                nc.sync.dma_start(xr[c][0:CIN, :], src)
                nc.scalar.dma_start(xr[c][CIN:2 * CIN, :], src)
                nc.gpsimd.dma_start(xr[c][2 * CIN:3 * CIN, :], src)
                if c == 0:
                    nc.sync.dma_start(wq[:], wt_d.ap())
                elif c == 1:
                    nc.sync.dma_start(aux[:], aux_d.ap())

            # quantize pass: pass1 on DVE over all 96 partitions; pass2
            # places groups g0/g1 (DVE) and g2 (Pool) into p96
            pool_q = {}   # chunk -> last Pool quant inst
            dve_insts = []
            for c in range(CHUNKS):
                nr = XNR[c]
                u = pool.tile([K96, nr * W], F32, name=f"u{c}", tag=f"u{c}")
                nc.vector.tensor_scalar(u[:], xr[c][:], float(inv_sx), MAGIC,
                                        MULT, ADD)
                for g in range(KW):
                    off = (XR0[c] + 1) * PW + 1 - g
                    dst = p96[g * CIN:(g + 1) * CIN, off:off + nr * PW] \
                        .rearrange("p (r c) -> p r c", c=PW)[:, :, 0:W]
                    srcv = u[g * CIN:(g + 1) * CIN, :].rearrange(
                        "p (r c) -> p r c", c=W)
                    # balance: g0 + (g1 on even chunks) -> DVE,
                    # g2 + (g1 on odd chunks) -> Pool
                    if g == 0 or (g == 1 and c % 2 == 0):
                        dve_insts.append(
                            nc.vector.tensor_scalar_add(dst, srcv, -MAGIC))
                    else:
                        pool_q[c] = nc.gpsimd.tensor_scalar_add(
                            dst, srcv, -MAGIC)

            # ---- conv: CHUNKS x 3 accumulating matmuls (K=96) ----
            for c in range(CHUNKS):
                ps = psum.tile([COUT, R * W], F32, name=f"ps{c}", tag=f"ps{c}")
                for kh in range(KH):
                    r0 = c * R + kh
                    rhs = p96_rows[:, r0:r0 + R, 0:W]
                    nc.tensor.matmul(
                        ps[:], wq[:, kh * COUT:(kh + 1) * COUT], rhs,
                        start=(kh == 0), stop=(kh == KH - 1))
                osb = pool.tile([COUT, R * W], F32, name=f"osb{c}",
                                tag=f"osb{c}")
                # GPSIMD can't read PSUM (walrus P2): epilogues go on ACT
                # (cheap PSUM access; table load hides in its idle window),
                # except the last one on DVE, which is free by then.
                if c == CHUNKS - 1:
                    epi = nc.vector.tensor_scalar(
                        osb[:], ps[:], float(s_out), bias, MULT, ADD)
                    tile.add_dep_helper(epi.ins, dve_insts[-1].ins,
                                        sync=False,
                                        reason="epilogue after DVE quant")
                else:
                    nc.scalar.activation(
                        osb[:], ps[:], mybir.ActivationFunctionType.Identity,
                        bias=bias, scale=float(s_out))
                eng = nc.scalar if c == CHUNKS - 1 else nc.sync
                eng.dma_start(out_d.ap()[:, c * R * W:(c + 1) * R * W], osb[:])

    nc.compile()
    return nc


def get_program(inv_sx, inv_sw, s_out):
    key = (float(inv_sx), float(inv_sw), float(s_out), CHUNKS)
    if key not in _CACHE:
        _CACHE[key] = _build_program(*key[:3])
    return _CACHE[key]


def _scales(x, weight):
    sx = np.float32(np.max(np.abs(x))) / np.float32(127.0)
    sw = np.float32(np.max(np.abs(weight))) / np.float32(127.0)
    inv_sx = np.float32(1.0) / sx
    inv_sw = np.float32(1.0) / sw
    return inv_sx, inv_sw, sx * sw


def make_in_maps(x, weight, bias, lut):
    import ml_dtypes
    x = np.asarray(x, dtype=np.float32)
    weight = np.asarray(weight, dtype=np.float32)
    bias = np.asarray(bias, dtype=np.float32)

    # host-quantized weights (tiny tensor; integer levels are exact in
    # bf16): wt[(g*32+cin), (kh*32+cout)] = round(weight/sw)[cout,cin,kh,g]
    _, inv_sw, _ = _scales(x, weight)
    qw = np.round(weight * inv_sw)
    wt = np.ascontiguousarray(
        qw.transpose(3, 1, 2, 0).reshape(K96, KH * COUT)).astype(
            ml_dtypes.bfloat16)
    aux = np.ascontiguousarray(bias.reshape(COUT, 1))

    return [
        {"x": np.ascontiguousarray(x[b].reshape(CIN, OHW)),
         "wt": wt, "aux": aux}
        for b in range(B)
    ]


def kernel(x, weight, bias, lut, **run_kwargs):
    x = np.asarray(x, dtype=np.float32)
    weight = np.asarray(weight, dtype=np.float32)
    nc = get_program(*_scales(x, weight))
    in_maps = make_in_maps(x, weight, bias, lut)
    res = run_bass_kernel_spmd(nc, in_maps, core_ids=list(range(N_CORES)),
                               **run_kwargs)
    out = np.stack([res.results[b]["out"].reshape(COUT, H, W)
                    for b in range(B)])
    _CACHE["last_results"] = res
    return out

